# revision 20
# baseline (speedup 1.0000x reference)
"""Trainium2 Bass kernel for nn_PolyAttention (16-head polynomial causal attention).

Reference math (fp32):
    q = x @ Wq.T; k = x @ Wk.T; v = x @ Wv.T        (per-head dim 128, 16 heads)
    q, k = rope(q), rope(k)                          (LRPE type-1, base 10000)
    s = (q . k)^4, causal-masked, row-normalized by max(sum, 1e-6)
    out = (s @ v normalized) @ Wo.T

Sharding: 8 cores = batch(2) x head-group(4 heads each).  Each core computes its
(b, head-group) shard end-to-end plus the Wo partial projection; the host sums
the 4 partials per batch element.

v2 design (vs v1):
  - all matmul operands bf16 (host-converted); fp32 PSUM accumulate; fp16 out.
    Numerics sim: rel_fro ~7e-3 (gate 2e-2).  bf16 halves DMA + SBUF and
    enables FWL fast weight loads.
  - single merged projection pass: x loaded once, q/k/v computed per n-chunk.
  - host relayouts inputs so every chunk/weight load is ONE dma_start with
    16KB contiguous per partition line.
  - attention: scores built transposed [keys, queries]; 2-block software
    pipeline (score chain runs 2 blocks ahead of the AV chain) so the PE
    never waits on the scalar/vector square/quartic pipeline.
  - denominator off the PE: DVE accumulates s4 blocks into sAcc, one gpsimd
    partition_all_reduce per (qb, h) replaces ones-matmuls + broadcast.
"""

import os
import sys

import numpy as np

if "/opt/trn_rl_repo" not in sys.path:
    sys.path.insert(0, "/opt/trn_rl_repo")

# ---------------------------------------------------------------- constants
B = 2
N = 2048
D = 2048
NH = 16
DH = 128
NHL = 4          # heads per core
HL = NHL * DH    # 512 local head dims
POLY = 4
EPS = 1e-6
LRPE_BASE = 10000.0

CH = 512         # projection n-chunk (columns of xT per step)
QB = 512         # query block
KB = 128         # key block
NDB = D // 128   # 16 contraction d-blocks
NCH = N // CH    # 4 chunks
NQB = N // QB    # 4 query blocks
NKBT = N // KB   # 16 key blocks total
LA = 4           # attention software-pipeline lookahead (score ahead of AV)


# ---------------------------------------------------------------- builder
def build_module(n=N):
    import concourse.bacc as bacc
    import concourse.mybir as mybir
    import concourse.tile as tile
    from concourse import bass_isa

    f32 = mybir.dt.float32
    bf16 = mybir.dt.bfloat16
    f16 = mybir.dt.float16
    AF = mybir.ActivationFunctionType

    nc = bacc.Bacc(
        "TRN2",
        target_bir_lowering=False,
        debug=False,
        enable_asserts=False,
        num_devices=8,
    )

    nch = n // CH
    nqb = n // QB

    # host-relayouted inputs (see make_in_maps)
    xt_d = nc.dram_tensor("xt", [128, nch, NDB * CH], bf16, kind="ExternalInput").ap()
    wq_d = nc.dram_tensor("wq", [128, NDB * HL], bf16, kind="ExternalInput").ap()
    wk_d = nc.dram_tensor("wk", [128, NDB * HL], bf16, kind="ExternalInput").ap()
    wv_d = nc.dram_tensor("wv", [128, NDB * HL], bf16, kind="ExternalInput").ap()
    wo_d = nc.dram_tensor("wo", [128, NHL * D], bf16, kind="ExternalInput").ap()
    cs_d = nc.dram_tensor("cs", [DH, n], f32, kind="ExternalInput").ap()
    sn_d = nc.dram_tensor("sn", [DH, n], f32, kind="ExternalInput").ap()
    mk_d = nc.dram_tensor("msk", [KB, KB], bf16, kind="ExternalInput").ap()
    out_d = nc.dram_tensor("out", [n, D], f16, kind="ExternalOutput").ap()

    def mm(out, lhsT, rhs, start, stop):
        nc.tensor.matmul(out, lhsT, rhs, start=start, stop=stop)

    with tile.TileContext(nc) as tc:
        from contextlib import ExitStack

        with ExitStack() as ctx:
            persist = ctx.enter_context(tc.tile_pool(name="persist", bufs=1))
            qTc = [persist.tile([128, NHL * CH], bf16, tag=f"qT{c}", name=f"qT{c}")
                   for c in range(nch)]
            kTc = [persist.tile([128, NHL * CH], bf16, tag=f"kT{c}", name=f"kT{c}")
                   for c in range(nch)]
            vSc = [persist.tile([128, (CH // 128) * HL], bf16, tag=f"vS{c}", name=f"vS{c}")
                   for c in range(nch)]
            nh2 = n // 2
            cs_t = [persist.tile([128, nh2], f32, tag=f"cs{i}", name=f"cs{i}") for i in range(2)]
            sn_t = [persist.tile([128, nh2], f32, tag=f"sn{i}", name=f"sn{i}") for i in range(2)]

            # PSUM: shps (2 banks) is shared by projection chains, Wo chains,
            # and 2 of every 5 score tiles; psS/psO/psD hold the other 6 banks.
            shps = ctx.enter_context(tc.tile_pool(name="shps", bufs=2, space="PSUM"))
            psS = ctx.enter_context(tc.tile_pool(name="c_ps", bufs=3, space="PSUM"))
            psO = ctx.enter_context(tc.tile_pool(name="c_po", bufs=2, space="PSUM"))
            psD = ctx.enter_context(tc.tile_pool(name="c_pd", bufs=1, space="PSUM"))

            wpool = ctx.enter_context(tc.tile_pool(name="w", bufs=1))
            xpool = ctx.enter_context(tc.tile_pool(name="x", bufs=2))
            tpool = ctx.enter_context(tc.tile_pool(name="t", bufs=2))
            s2pool = ctx.enter_context(tc.tile_pool(name="s2", bufs=4))
            s4pool = ctx.enter_context(tc.tile_pool(name="s4", bufs=6))
            rbpool = ctx.enter_context(tc.tile_pool(name="rb", bufs=2))
            onpool = ctx.enter_context(tc.tile_pool(name="on", bufs=2))
            fopool = ctx.enter_context(tc.tile_pool(name="fo", bufs=2))

            wq_t = wpool.tile([128, NDB * HL], bf16, tag="wq", name="wq")
            wk_t = wpool.tile([128, NDB * HL], bf16, tag="wk", name="wk")
            wv_t = wpool.tile([128, NDB * HL], bf16, tag="wv", name="wv")
            wo_t = wpool.tile([128, NHL * D], bf16, tag="wo", name="wo")
            mk = wpool.tile([128, KB], bf16, tag="mk", name="mk")
            ones = wpool.tile([128, 1], bf16, tag="ones", name="ones")
            epsv = wpool.tile([1, 1], bf16, tag="epsv", name="epsv")
            oner = wpool.tile([1, QB], bf16, tag="oner", name="oner")
            nc.vector.memset(ones[:], 1.0)
            nc.vector.memset(epsv[:], EPS)
            nc.vector.memset(oner[:], 1.0)
            xt_cs = [xpool.tile([128, NDB * CH], bf16, tag="xtc", name="xtc")
                     for c in range(nch)]

            # startup DMA order = first-use order, quarter-granular up front
            qtr = NDB * HL // 4
            half = NDB * HL // 2
            for p in range(4):
                nc.sync.dma_start(wq_t[:, p * qtr:(p + 1) * qtr], wq_d[:, p * qtr:(p + 1) * qtr])
                nc.sync.dma_start(xt_cs[0][:, p * qtr:(p + 1) * qtr], xt_d[:, 0, p * qtr:(p + 1) * qtr])
                if p == 0:
                    nc.sync.dma_start(cs_t[0][:, 0:CH], cs_d[:, 0:CH])
                    nc.sync.dma_start(sn_t[0][:, 0:CH], sn_d[:, 0:CH])
            for p in range(2):
                nc.sync.dma_start(wk_t[:, p * half:(p + 1) * half], wk_d[:, p * half:(p + 1) * half])
            nc.sync.dma_start(mk[:], mk_d[:, :])
            for p in range(2):
                nc.sync.dma_start(wv_t[:, p * half:(p + 1) * half], wv_d[:, p * half:(p + 1) * half])
            nc.sync.dma_start(cs_t[0][:, CH:], cs_d[:, CH:nh2])
            nc.sync.dma_start(sn_t[0][:, CH:], sn_d[:, CH:nh2])
            for p in range(2):
                nc.sync.dma_start(wo_t[:, p * NHL * D // 2:(p + 1) * NHL * D // 2],
                                  wo_d[:, p * NHL * D // 2:(p + 1) * NHL * D // 2])
            nc.sync.dma_start(cs_t[1][:], cs_d[:, nh2:])
            nc.sync.dma_start(sn_t[1][:], sn_d[:, nh2:])

            pss_ctr = [0]

            def pss_tile():
                i = pss_ctr[0]
                pss_ctr[0] += 1
                if i % 5 < 3:
                    return psS.tile([128, QB], f32, tag="pss", name="pss")
                return shps.tile([128, QB], f32, tag="ps", name="pss")

            def chunk_dma(c):
                xt_c = xt_cs[c]
                if c > 0:
                    nc.sync.dma_start(xt_c[:, 0:half], xt_d[:, c, 0:half])
                    nc.sync.dma_start(xt_c[:, half:], xt_d[:, c, half:])

            def chunk_chains(c):
                """Return the 12 projection-chain emitters for chunk c."""
                c0 = c * CH
                xt_c = xt_cs[c]
                csh = cs_t[(c0 // nh2)][:, c0 % nh2: c0 % nh2 + CH]
                snh = sn_t[(c0 // nh2)][:, c0 % nh2: c0 % nh2 + CH]
                out = []

                def qk_chain(w_t, dstT, h):
                    ps = shps.tile([128, CH], f32, tag="ps", name="ps")
                    for i in range(NDB):
                        mm(ps[:], w_t[:, i * HL + h * 128: i * HL + (h + 1) * 128],
                           xt_c[:, i * CH:(i + 1) * CH],
                           start=(i == 0), stop=(i == NDB - 1))
                    dst = dstT[:, h * CH:(h + 1) * CH]
                    swp = tpool.tile([128, CH], f32, tag="swp", name="swp")
                    nc.scalar.copy(swp[0:64, :], ps[64:128, :])
                    nc.scalar.copy(swp[64:128, :], ps[0:64, :])
                    m1 = tpool.tile([128, CH], f32, tag="m1", name="m1")
                    nc.vector.tensor_mul(m1[:], ps[:], csh)
                    m2 = tpool.tile([128, CH], f32, tag="m2", name="m2")
                    nc.vector.tensor_mul(m2[:], swp[:], snh)
                    nc.vector.tensor_add(dst, m1[:], m2[:])

                def v_chain(t2):
                    psv = shps.tile([128, HL], f32, tag="ps", name="psv")
                    for i in range(NDB):
                        mm(psv[:], xt_c[:, i * CH + t2 * 128: i * CH + (t2 + 1) * 128],
                           wv_t[:, i * HL:(i + 1) * HL],
                           start=(i == 0), stop=(i == NDB - 1))
                    nc.scalar.copy(vSc[c][:, t2 * HL:(t2 + 1) * HL], psv[:])

                for h in range(NHL):
                    out.append(lambda h=h: qk_chain(wq_t, qTc[c], h))
                for h in range(NHL):
                    out.append(lambda h=h: qk_chain(wk_t, kTc[c], h))
                for t2 in range(CH // 128):
                    out.append(lambda t2=t2: v_chain(t2))
                return out

            pend = []       # deferred normalize-mul emission (cross-qb)
            wo_pend = []    # deferred per-qt Wo emitters from the previous qb

            def emit_attention(qb, fillers):
                nkb = (qb + 1) * (QB // KB)
                steps_total = NHL * (nkb + LA)
                nf = len(fillers)
                fi = [0]
                sg = [0]

                def tick():
                    while fi[0] < nf and sg[0] >= (fi[0] + 1) * steps_total // (nf + 1):
                        fillers[fi[0]]()
                        fi[0] += 1
                    sg[0] += 1
                onrm = [onpool.tile([128, QB], bf16, tag=f"onrm{h}", name=f"onrm{h}")
                        for h in range(NHL)]
                for h in range(NHL):
                    pso = psO.tile([128, QB], f32, tag="pso", name="pso")
                    psd = psD.tile([1, QB], f32, tag="psd", name="psd")
                    s4q = {}
                    for step in range(nkb + LA):
                        if step == 1 and pend:
                            pend.pop()()
                        if step == 2 and wo_pend:
                            wo_pend.pop(0)()
                        tick()
                        if step < nkb:
                            kb = step
                            rel = kb - qb * (QB // KB)
                            cr = 0 if rel < 0 else 128 * rel
                            pss = pss_tile()
                            kc, kr = kb // (CH // KB), kb % (CH // KB)
                            mm(pss[:, cr:],
                               kTc[kc][:, h * CH + kr * KB: h * CH + (kr + 1) * KB],
                               qTc[qb][:, h * CH + cr:(h + 1) * CH],
                               start=True, stop=True)
                            s2 = s2pool.tile([128, QB], bf16, tag="s2", name="s2")
                            nc.scalar.activation(s2[:, cr:], pss[:, cr:], AF.Square)
                            if rel >= 0:
                                nc.vector.tensor_mul(s2[:, cr:cr + 128],
                                                     s2[:, cr:cr + 128], mk[:])
                            s4 = s4pool.tile([128, QB], bf16, tag="s4", name="s4")
                            nc.vector.tensor_mul(s4[:, cr:], s2[:, cr:], s2[:, cr:])
                            s4q[kb] = (s4, cr)
                        if step >= LA:
                            kb = step - LA
                            s4, cr = s4q.pop(kb)
                            kc, kr = kb // (CH // KB), kb % (CH // KB)
                            mm(pso[:, cr:],
                               vSc[kc][:, kr * HL + h * 128: kr * HL + (h + 1) * 128],
                               s4[:, cr:],
                               start=(kb == 0), stop=(kb == nkb - 1))
                            mm(psd[0:1, cr:], ones[:, 0:1], s4[:, cr:],
                               start=(kb == 0), stop=False)
                    # + eps, so the reciprocal input is strictly positive
                    # (row-0 denominators are >=7e-3 here, so +eps == max(,eps))
                    mm(psd[0:1, :], epsv[0:1, 0:1], oner[0:1, :],
                       start=False, stop=True)
                    rbr = rbpool.tile([1, QB], f32, tag="rbr", name="rbr")
                    nc.vector.reciprocal_approx_fast(rbr[:], psd[0:1, :])
                    rbc = rbpool.tile([128, QB], f32, tag="rbc", name="rbc")
                    nc.gpsimd.partition_broadcast(rbc[:], rbr[:])

                    def _norm(h=h, pso=pso, rbc=rbc, onrm=onrm):
                        nc.vector.tensor_mul(onrm[h][:], pso[:], rbc[:])
                    pend.append(_norm)

                # Wo chains for this qb run as PE filler during the NEXT qb's
                # attention (ACT-paced), overlapping the two phases.  One
                # closure per qt row-block (4 chains + copies + store).
                def emit_wo_qt(qt, qb=qb, onrm=onrm):
                    fout = fopool.tile([128, D], f16, tag="fout", name="fout")
                    r0 = qb * QB + qt * 128
                    for jc in range(D // 512):
                        psf = shps.tile([128, 512], f32, tag="ps", name="psf")
                        for h in range(NHL):
                            mm(psf[:], onrm[h][:, qt * 128:(qt + 1) * 128],
                               wo_t[:, h * D + jc * 512: h * D + (jc + 1) * 512],
                               start=(h == 0), stop=(h == NHL - 1))
                        if jc % 2 == 0:
                            nc.scalar.copy(fout[:, jc * 512:(jc + 1) * 512], psf[:])
                        else:
                            nc.vector.tensor_copy(fout[:, jc * 512:(jc + 1) * 512], psf[:])
                        if jc == 1:
                            nc.sync.dma_start(out_d[r0:r0 + 128, 0:1024], fout[:, 0:1024])
                    nc.sync.dma_start(out_d[r0:r0 + 128, 1024:], fout[:, 1024:])

                for qt in range(QB // 128):
                    wo_pend.append(lambda qt=qt: emit_wo_qt(qt))

            for c in range(nch):
                for f in chunk_chains(c):
                    f()
                if c + 1 < nch:
                    chunk_dma(c + 1)
                emit_attention(c, [])
            while pend:
                pend.pop()()
            while wo_pend:
                wo_pend.pop(0)()

    nc.compile()
    return nc


# ---------------------------------------------------------------- host prep
def _rope_tables(n):
    half = DH // 2
    theta = LRPE_BASE ** (-np.arange(half, dtype=np.float64) * 2.0 / DH)
    pos = np.arange(n, dtype=np.float64)
    ang = np.outer(pos, theta)                       # [n, 64]
    cos = np.cos(ang).T.astype(np.float32)           # [64, n]
    sin = np.sin(ang).T.astype(np.float32)
    cs = np.concatenate([cos, cos], axis=0)          # [128, n]
    sn = np.concatenate([-sin, sin], axis=0)
    return np.ascontiguousarray(cs), np.ascontiguousarray(sn)


def _mask():
    kp = np.arange(KB)[:, None]
    j = np.arange(KB)[None, :]
    return (kp <= j).astype(np.float32)


def make_in_maps(x, Wq, Wk, Wv, Wo, n=N):
    import ml_dtypes
    bf16 = ml_dtypes.bfloat16

    cs, sn = _rope_tables(n)
    mk = _mask()
    nch = n // CH

    def relayout_x(xb):
        # xt [d, n] -> [128, nch, NDB*CH]: chunk c contiguous per partition
        xt = np.ascontiguousarray(xb.T)
        a = xt.reshape(NDB, 128, nch, CH).transpose(1, 2, 0, 3)
        return np.ascontiguousarray(a.reshape(128, nch, NDB * CH).astype(bf16))

    def relayout_w(Wrows):
        # W[rows,:].T [d, HL] -> [128, NDB*HL]
        w = Wrows.T.reshape(NDB, 128, HL).transpose(1, 0, 2)
        return np.ascontiguousarray(w.reshape(128, NDB * HL).astype(bf16))

    def relayout_wo(Wcols):
        # Wo[:, rows].T [HL, D] -> [128, NHL*D]
        w = Wcols.T.reshape(NHL, 128, D).transpose(1, 0, 2)
        return np.ascontiguousarray(w.reshape(128, NHL * D).astype(bf16))

    xts = [relayout_x(x[b]) for b in range(x.shape[0])]
    in_maps = []
    for core in range(8):
        b, g = core // 4, core % 4
        rows = slice(g * HL, (g + 1) * HL)
        in_maps.append({
            "xt": xts[b],
            "wq": relayout_w(Wq[rows, :]),
            "wk": relayout_w(Wk[rows, :]),
            "wv": relayout_w(Wv[rows, :]),
            "wo": relayout_wo(Wo[:, rows]),
            "cs": cs,
            "sn": sn,
            "msk": mk.astype(bf16),
        })
    return in_maps


_NC_CACHE = {}


def _get_nc(n=N):
    if n not in _NC_CACHE:
        _NC_CACHE[n] = build_module(n)
    return _NC_CACHE[n]


def run(x, Wq, Wk, Wv, Wo, trace=False, **kw):
    from concourse.bass_utils import run_bass_kernel_spmd

    x = np.asarray(x, dtype=np.float32)
    nc = _get_nc(x.shape[1])
    in_maps = make_in_maps(x, Wq, Wk, Wv, Wo, n=x.shape[1])
    res = run_bass_kernel_spmd(nc, in_maps, core_ids=list(range(8)), trace=trace, **kw)
    outs = [np.asarray(res.results[i]["out"], dtype=np.float32) for i in range(8)]
    b0 = outs[0] + outs[1] + outs[2] + outs[3]
    b1 = outs[4] + outs[5] + outs[6] + outs[7]
    out = np.stack([b0, b1]).astype(np.float32)
    return out, res


def kernel(x, Wq, Wk, Wv, Wo):
    out, _ = run(
        np.asarray(x, np.float32),
        np.asarray(Wq, np.float32),
        np.asarray(Wk, np.float32),
        np.asarray(Wv, np.float32),
        np.asarray(Wo, np.float32),
    )
    return out


# revision 21
# speedup vs baseline: 1.1890x; 1.1890x over previous
"""Trainium2 Bass kernel for nn_PolyAttention (16-head polynomial causal attention).

Reference math (fp32):
    q = x @ Wq.T; k = x @ Wk.T; v = x @ Wv.T        (per-head dim 128, 16 heads)
    q, k = rope(q), rope(k)                          (LRPE type-1, base 10000)
    s = (q . k)^4, causal-masked, row-normalized by max(sum, 1e-6)
    out = (s @ v normalized) @ Wo.T

Sharding: 8 cores = batch(2) x head-group(4 heads each).  Each core computes its
(b, head-group) shard end-to-end plus the Wo partial projection; the host sums
the 4 partials per batch element.

v2 design (vs v1):
  - all matmul operands bf16 (host-converted); fp32 PSUM accumulate; fp16 out.
    Numerics sim: rel_fro ~7e-3 (gate 2e-2).  bf16 halves DMA + SBUF and
    enables FWL fast weight loads.
  - single merged projection pass: x loaded once, q/k/v computed per n-chunk.
  - host relayouts inputs so every chunk/weight load is ONE dma_start with
    16KB contiguous per partition line.
  - attention: scores built transposed [keys, queries]; 2-block software
    pipeline (score chain runs 2 blocks ahead of the AV chain) so the PE
    never waits on the scalar/vector square/quartic pipeline.
  - denominator off the PE: DVE accumulates s4 blocks into sAcc, one gpsimd
    partition_all_reduce per (qb, h) replaces ones-matmuls + broadcast.
"""

import os
import sys

import numpy as np

if "/opt/trn_rl_repo" not in sys.path:
    sys.path.insert(0, "/opt/trn_rl_repo")

# ---------------------------------------------------------------- constants
B = 2
N = 2048
D = 2048
NH = 16
DH = 128
NHL = 4          # heads per core
HL = NHL * DH    # 512 local head dims
POLY = 4
EPS = 1e-6
LRPE_BASE = 10000.0

CH = 512         # projection n-chunk (columns of xT per step)
QB = 512         # query block
KB = 128         # key block
NDB = D // 128   # 16 contraction d-blocks
NCH = N // CH    # 4 chunks
NQB = N // QB    # 4 query blocks
NKBT = N // KB   # 16 key blocks total
LA = 4           # attention software-pipeline lookahead (score ahead of AV)


# ---------------------------------------------------------------- builder
def build_module(n=N):
    import concourse.bacc as bacc
    import concourse.mybir as mybir
    import concourse.tile as tile
    from concourse import bass_isa

    f32 = mybir.dt.float32
    bf16 = mybir.dt.bfloat16
    f16 = mybir.dt.float16
    AF = mybir.ActivationFunctionType

    nc = bacc.Bacc(
        "TRN2",
        target_bir_lowering=False,
        debug=False,
        enable_asserts=False,
        num_devices=8,
    )

    nch = n // CH
    nqb = n // QB

    # host-relayouted inputs (see make_in_maps)
    xt_d = nc.dram_tensor("xt", [128, nch, NDB * CH], bf16, kind="ExternalInput").ap()
    wq_d = nc.dram_tensor("wq", [128, NDB * HL], bf16, kind="ExternalInput").ap()
    wk_d = nc.dram_tensor("wk", [128, NDB * HL], bf16, kind="ExternalInput").ap()
    wv_d = nc.dram_tensor("wv", [128, NDB * HL], bf16, kind="ExternalInput").ap()
    wo_d = nc.dram_tensor("wo", [128, NHL * D], bf16, kind="ExternalInput").ap()
    cs_d = nc.dram_tensor("cs", [DH, n], f32, kind="ExternalInput").ap()
    sn_d = nc.dram_tensor("sn", [DH, n], f32, kind="ExternalInput").ap()
    mk_d = nc.dram_tensor("msk", [KB, KB], bf16, kind="ExternalInput").ap()
    out_d = nc.dram_tensor("out", [n, D], f16, kind="ExternalOutput").ap()

    def mm(out, lhsT, rhs, start, stop):
        nc.tensor.matmul(out, lhsT, rhs, start=start, stop=stop)

    with tile.TileContext(nc) as tc:
        from contextlib import ExitStack

        with ExitStack() as ctx:
            persist = ctx.enter_context(tc.tile_pool(name="persist", bufs=1))
            qTc = [persist.tile([128, NHL * CH], bf16, tag=f"qT{c}", name=f"qT{c}")
                   for c in range(nch)]
            kTc = [persist.tile([128, NHL * CH], bf16, tag=f"kT{c}", name=f"kT{c}")
                   for c in range(nch)]
            vSc = [persist.tile([128, (CH // 128) * HL], bf16, tag=f"vS{c}", name=f"vS{c}")
                   for c in range(nch)]
            nh2 = n // 2
            cs_t = [persist.tile([128, nh2], f32, tag=f"cs{i}", name=f"cs{i}") for i in range(2)]
            sn_t = [persist.tile([128, nh2], f32, tag=f"sn{i}", name=f"sn{i}") for i in range(2)]

            # PSUM: shps (2 banks) is shared by projection chains, Wo chains,
            # and 2 of every 5 score tiles; psS/psO/psD hold the other 6 banks.
            shps = ctx.enter_context(tc.tile_pool(name="shps", bufs=2, space="PSUM"))
            psS = ctx.enter_context(tc.tile_pool(name="c_ps", bufs=3, space="PSUM"))
            psO = ctx.enter_context(tc.tile_pool(name="c_po", bufs=2, space="PSUM"))
            psD = ctx.enter_context(tc.tile_pool(name="c_pd", bufs=1, space="PSUM"))

            wpool = ctx.enter_context(tc.tile_pool(name="w", bufs=1))
            xpool = ctx.enter_context(tc.tile_pool(name="x", bufs=2))
            tpool = ctx.enter_context(tc.tile_pool(name="t", bufs=2))
            s2pool = ctx.enter_context(tc.tile_pool(name="s2", bufs=4))
            s4pool = ctx.enter_context(tc.tile_pool(name="s4", bufs=6))
            rbpool = ctx.enter_context(tc.tile_pool(name="rb", bufs=2))
            onpool = ctx.enter_context(tc.tile_pool(name="on", bufs=2))
            fopool = ctx.enter_context(tc.tile_pool(name="fo", bufs=2))

            wq_t = wpool.tile([128, NDB * HL], bf16, tag="wq", name="wq")
            wk_t = wpool.tile([128, NDB * HL], bf16, tag="wk", name="wk")
            wv_t = wpool.tile([128, NDB * HL], bf16, tag="wv", name="wv")
            wo_t = wpool.tile([128, NHL * D], bf16, tag="wo", name="wo")
            mk = wpool.tile([128, KB], bf16, tag="mk", name="mk")
            ones = wpool.tile([128, 1], bf16, tag="ones", name="ones")
            epsv = wpool.tile([1, 1], bf16, tag="epsv", name="epsv")
            oner = wpool.tile([1, QB], bf16, tag="oner", name="oner")
            nc.vector.memset(ones[:], 1.0)
            nc.vector.memset(epsv[:], EPS)
            nc.vector.memset(oner[:], 1.0)
            xt_cs = [xpool.tile([128, NDB * CH], bf16, tag="xtc", name="xtc")
                     for c in range(nch)]

            # startup DMA order = first-use order, quarter-granular up front
            qtr = NDB * HL // 4
            half = NDB * HL // 2
            for p in range(4):
                nc.sync.dma_start(wq_t[:, p * qtr:(p + 1) * qtr], wq_d[:, p * qtr:(p + 1) * qtr])
                nc.sync.dma_start(xt_cs[0][:, p * qtr:(p + 1) * qtr], xt_d[:, 0, p * qtr:(p + 1) * qtr])
                if p == 0:
                    nc.sync.dma_start(cs_t[0][:, 0:CH], cs_d[:, 0:CH])
                    nc.sync.dma_start(sn_t[0][:, 0:CH], sn_d[:, 0:CH])
            for p in range(2):
                nc.sync.dma_start(wk_t[:, p * half:(p + 1) * half], wk_d[:, p * half:(p + 1) * half])
            nc.sync.dma_start(mk[:], mk_d[:, :])
            for p in range(2):
                nc.sync.dma_start(wv_t[:, p * half:(p + 1) * half], wv_d[:, p * half:(p + 1) * half])
            nc.sync.dma_start(cs_t[0][:, CH:], cs_d[:, CH:nh2])
            nc.sync.dma_start(sn_t[0][:, CH:], sn_d[:, CH:nh2])
            for p in range(2):
                nc.sync.dma_start(wo_t[:, p * NHL * D // 2:(p + 1) * NHL * D // 2],
                                  wo_d[:, p * NHL * D // 2:(p + 1) * NHL * D // 2])
            nc.sync.dma_start(cs_t[1][:], cs_d[:, nh2:])
            nc.sync.dma_start(sn_t[1][:], sn_d[:, nh2:])

            pss_ctr = [0]

            def pss_tile():
                i = pss_ctr[0]
                pss_ctr[0] += 1
                if i % 5 < 3:
                    return psS.tile([128, QB], f32, tag="pss", name="pss")
                return shps.tile([128, QB], f32, tag="ps", name="pss")

            def chunk_dma(c):
                xt_c = xt_cs[c]
                if c > 0:
                    nc.sync.dma_start(xt_c[:, 0:half], xt_d[:, c, 0:half])
                    nc.sync.dma_start(xt_c[:, half:], xt_d[:, c, half:])

            def chunk_chains(c):
                """Return the 12 projection-chain emitters for chunk c."""
                c0 = c * CH
                xt_c = xt_cs[c]
                csh = cs_t[(c0 // nh2)][:, c0 % nh2: c0 % nh2 + CH]
                snh = sn_t[(c0 // nh2)][:, c0 % nh2: c0 % nh2 + CH]
                out = []

                def qk_chain(w_t, dstT, h):
                    ps = shps.tile([128, CH], f32, tag="ps", name="ps")
                    for i in range(NDB):
                        mm(ps[:], w_t[:, i * HL + h * 128: i * HL + (h + 1) * 128],
                           xt_c[:, i * CH:(i + 1) * CH],
                           start=(i == 0), stop=(i == NDB - 1))
                    dst = dstT[:, h * CH:(h + 1) * CH]
                    swp = tpool.tile([128, CH], f32, tag="swp", name="swp")
                    nc.scalar.copy(swp[0:64, :], ps[64:128, :])
                    nc.scalar.copy(swp[64:128, :], ps[0:64, :])
                    m1 = tpool.tile([128, CH], f32, tag="m1", name="m1")
                    nc.vector.tensor_mul(m1[:], ps[:], csh)
                    m2 = tpool.tile([128, CH], f32, tag="m2", name="m2")
                    nc.vector.tensor_mul(m2[:], swp[:], snh)
                    nc.vector.tensor_add(dst, m1[:], m2[:])

                def v_chain(t2):
                    psv = shps.tile([128, HL], f32, tag="ps", name="psv")
                    for i in range(NDB):
                        mm(psv[:], xt_c[:, i * CH + t2 * 128: i * CH + (t2 + 1) * 128],
                           wv_t[:, i * HL:(i + 1) * HL],
                           start=(i == 0), stop=(i == NDB - 1))
                    nc.scalar.copy(vSc[c][:, t2 * HL:(t2 + 1) * HL], psv[:])

                for h in range(NHL):
                    out.append(lambda h=h: qk_chain(wq_t, qTc[c], h))
                for h in range(NHL):
                    out.append(lambda h=h: qk_chain(wk_t, kTc[c], h))
                for t2 in range(CH // 128):
                    out.append(lambda t2=t2: v_chain(t2))
                return out

            pend = []       # deferred normalize-mul emission (cross-qb)
            wo_pend = []    # deferred per-qt Wo emitters from the previous qb

            def emit_attention(qb, fillers):
                nkb = (qb + 1) * (QB // KB)
                steps_total = NHL * (nkb + LA)
                nf = len(fillers)
                fi = [0]
                sg = [0]

                def tick():
                    while fi[0] < nf and sg[0] >= (fi[0] + 1) * steps_total // (nf + 1):
                        fillers[fi[0]]()
                        fi[0] += 1
                    sg[0] += 1
                onrm = [onpool.tile([128, QB], bf16, tag=f"onrm{h}", name=f"onrm{h}")
                        for h in range(NHL)]
                for h in range(NHL):
                    pso = psO.tile([128, QB], f32, tag="pso", name="pso")
                    psd = psD.tile([1, QB], f32, tag="psd", name="psd")
                    s4q = {}
                    for step in range(nkb + LA):
                        if step == 1 and pend:
                            pend.pop()()
                        if step == 2 and wo_pend:
                            wo_pend.pop(0)()
                        tick()
                        if step < nkb:
                            kb = step
                            rel = kb - qb * (QB // KB)
                            cr = 0 if rel < 0 else 128 * rel
                            pss = pss_tile()
                            kc, kr = kb // (CH // KB), kb % (CH // KB)
                            mm(pss[:, cr:],
                               kTc[kc][:, h * CH + kr * KB: h * CH + (kr + 1) * KB],
                               qTc[qb][:, h * CH + cr:(h + 1) * CH],
                               start=True, stop=True)
                            s2 = s2pool.tile([128, QB], bf16, tag="s2", name="s2")
                            nc.scalar.activation(s2[:, cr:], pss[:, cr:], AF.Square)
                            if rel >= 0:
                                nc.vector.tensor_mul(s2[:, cr:cr + 128],
                                                     s2[:, cr:cr + 128], mk[:])
                            s4 = s4pool.tile([128, QB], bf16, tag="s4", name="s4")
                            nc.vector.tensor_mul(s4[:, cr:], s2[:, cr:], s2[:, cr:])
                            s4q[kb] = (s4, cr)
                        if step >= LA:
                            kb = step - LA
                            s4, cr = s4q.pop(kb)
                            kc, kr = kb // (CH // KB), kb % (CH // KB)
                            mm(pso[:, cr:],
                               vSc[kc][:, kr * HL + h * 128: kr * HL + (h + 1) * 128],
                               s4[:, cr:],
                               start=(kb == 0), stop=(kb == nkb - 1))
                            mm(psd[0:1, cr:], ones[:, 0:1], s4[:, cr:],
                               start=(kb == 0), stop=False)
                    # + eps, so the reciprocal input is strictly positive
                    # (row-0 denominators are >=7e-3 here, so +eps == max(,eps))
                    mm(psd[0:1, :], epsv[0:1, 0:1], oner[0:1, :],
                       start=False, stop=True)
                    rbr = rbpool.tile([1, QB], f32, tag="rbr", name="rbr")
                    nc.vector.reciprocal_approx_fast(rbr[:], psd[0:1, :])
                    rbc = rbpool.tile([128, QB], f32, tag="rbc", name="rbc")
                    nc.gpsimd.partition_broadcast(rbc[:], rbr[:])

                    def _norm(h=h, pso=pso, rbc=rbc, onrm=onrm):
                        nc.vector.tensor_mul(onrm[h][:], pso[:], rbc[:])
                    pend.append(_norm)

                # Wo chains for this qb run as PE filler during the NEXT qb's
                # attention (ACT-paced), overlapping the two phases.  One
                # closure per qt row-block (4 chains + copies + store).
                def emit_wo_qt(qt, qb=qb, onrm=onrm):
                    fout = fopool.tile([128, D], f16, tag="fout", name="fout")
                    for jc in range(D // 512):
                        psf = shps.tile([128, 512], f32, tag="ps", name="psf")
                        for h in range(NHL):
                            mm(psf[:], onrm[h][:, qt * 128:(qt + 1) * 128],
                               wo_t[:, h * D + jc * 512: h * D + (jc + 1) * 512],
                               start=(h == 0), stop=(h == NHL - 1))
                        if jc % 2 == 0:
                            nc.scalar.copy(fout[:, jc * 512:(jc + 1) * 512], psf[:])
                        else:
                            nc.vector.tensor_copy(fout[:, jc * 512:(jc + 1) * 512], psf[:])
                    r0 = qb * QB + qt * 128
                    nc.sync.dma_start(out_d[r0:r0 + 128, :], fout[:])

                for qt in range(QB // 128):
                    wo_pend.append(lambda qt=qt: emit_wo_qt(qt))

            for c in range(nch):
                for f in chunk_chains(c):
                    f()
                if c + 1 < nch:
                    chunk_dma(c + 1)
                emit_attention(c, [])
            while pend:
                pend.pop()()
            while wo_pend:
                wo_pend.pop(0)()

    nc.compile()
    return nc


# ---------------------------------------------------------------- host prep
def _rope_tables(n):
    half = DH // 2
    theta = LRPE_BASE ** (-np.arange(half, dtype=np.float64) * 2.0 / DH)
    pos = np.arange(n, dtype=np.float64)
    ang = np.outer(pos, theta)                       # [n, 64]
    cos = np.cos(ang).T.astype(np.float32)           # [64, n]
    sin = np.sin(ang).T.astype(np.float32)
    cs = np.concatenate([cos, cos], axis=0)          # [128, n]
    sn = np.concatenate([-sin, sin], axis=0)
    return np.ascontiguousarray(cs), np.ascontiguousarray(sn)


def _mask():
    kp = np.arange(KB)[:, None]
    j = np.arange(KB)[None, :]
    return (kp <= j).astype(np.float32)


def make_in_maps(x, Wq, Wk, Wv, Wo, n=N):
    import ml_dtypes
    bf16 = ml_dtypes.bfloat16

    cs, sn = _rope_tables(n)
    mk = _mask()
    nch = n // CH

    def relayout_x(xb):
        # xt [d, n] -> [128, nch, NDB*CH]: chunk c contiguous per partition
        xt = np.ascontiguousarray(xb.T)
        a = xt.reshape(NDB, 128, nch, CH).transpose(1, 2, 0, 3)
        return np.ascontiguousarray(a.reshape(128, nch, NDB * CH).astype(bf16))

    def relayout_w(Wrows):
        # W[rows,:].T [d, HL] -> [128, NDB*HL]
        w = Wrows.T.reshape(NDB, 128, HL).transpose(1, 0, 2)
        return np.ascontiguousarray(w.reshape(128, NDB * HL).astype(bf16))

    def relayout_wo(Wcols):
        # Wo[:, rows].T [HL, D] -> [128, NHL*D]
        w = Wcols.T.reshape(NHL, 128, D).transpose(1, 0, 2)
        return np.ascontiguousarray(w.reshape(128, NHL * D).astype(bf16))

    xts = [relayout_x(x[b]) for b in range(x.shape[0])]
    in_maps = []
    for core in range(8):
        b, g = core // 4, core % 4
        rows = slice(g * HL, (g + 1) * HL)
        in_maps.append({
            "xt": xts[b],
            "wq": relayout_w(Wq[rows, :]),
            "wk": relayout_w(Wk[rows, :]),
            "wv": relayout_w(Wv[rows, :]),
            "wo": relayout_wo(Wo[:, rows]),
            "cs": cs,
            "sn": sn,
            "msk": mk.astype(bf16),
        })
    return in_maps


_NC_CACHE = {}


def _get_nc(n=N):
    if n not in _NC_CACHE:
        _NC_CACHE[n] = build_module(n)
    return _NC_CACHE[n]


def run(x, Wq, Wk, Wv, Wo, trace=False, **kw):
    from concourse.bass_utils import run_bass_kernel_spmd

    x = np.asarray(x, dtype=np.float32)
    nc = _get_nc(x.shape[1])
    in_maps = make_in_maps(x, Wq, Wk, Wv, Wo, n=x.shape[1])
    res = run_bass_kernel_spmd(nc, in_maps, core_ids=list(range(8)), trace=trace, **kw)
    outs = [np.asarray(res.results[i]["out"], dtype=np.float32) for i in range(8)]
    b0 = outs[0] + outs[1] + outs[2] + outs[3]
    b1 = outs[4] + outs[5] + outs[6] + outs[7]
    out = np.stack([b0, b1]).astype(np.float32)
    return out, res


def kernel(x, Wq, Wk, Wv, Wo):
    out, _ = run(
        np.asarray(x, np.float32),
        np.asarray(Wq, np.float32),
        np.asarray(Wk, np.float32),
        np.asarray(Wv, np.float32),
        np.asarray(Wo, np.float32),
    )
    return out
